# revision 1
# baseline (speedup 1.0000x reference)
"""BrainGAT (2x GATv2Conv + residuals + FC) on 8 Trainium2 NeuronCores.

Sharding: nodes partitioned across 8 cores (2500 each); edges assigned to the
core owning their destination, sorted by destination, processed in
125-node destination blocks. Small weights replicated. Per-layer source
features exchanged with one AllGather.

Per-edge math (exact softmax rewrite): with m_d = self-loop logit of node d,
alpha_e = exp(logit_e - m_d) / (sum_e' exp(logit_e' - m_d) + 1), and the
appended self-loop contributes exp(0)=1 and xl_d to the segment sums, so
self-loops never enter the edge lists.
"""
import numpy as np

import concourse.bass as bass
import concourse.bacc as bacc
import concourse.mybir as mybir
import concourse.tile as tile
from concourse.bass_utils import run_bass_kernel_spmd
from concourse.masks import make_identity

f32 = mybir.dt.float32
bf16 = mybir.dt.bfloat16
f32r = mybir.dt.float32r
i32 = mybir.dt.int32
NPBF = mybir.dt.np(bf16)
AF = mybir.ActivationFunctionType
ALU = mybir.AluOpType

NC = 8
HEADS = 4
NEG_SLOPE = 0.2


# ----------------------------------------------------------------------------
# device program builder
# ----------------------------------------------------------------------------

def build_program(N, B, NB, NT, in_ch, c1, c2, out_ch, dbg=False):
    """N: total nodes; B: dst-block size (<=125); NB: blocks/core;
    NT: 128-edge tiles per block; channel sizes as in the model."""
    npc = B * NB                  # nodes per core
    hc1 = HEADS * c1
    hc2 = HEADS * c2
    ntt = NB * NT                 # edge tiles per core per layer
    P = 128

    nc = bacc.Bacc("TRN2", target_bir_lowering=False, debug=False)

    def inp(name, shape, dt=f32):
        return nc.dram_tensor(name, shape, dt, kind="ExternalInput")

    xT = inp("xT", [in_ch, npc], f32r)            # own x, transposed
    src1 = inp("src1", [P, ntt], i32)             # gather indices, layer 1
    dst1 = inp("dst1", [P, ntt], bf16)            # local dst, layer 1
    dstrow = inp("dstrow", [ntt, P], bf16)        # same, one row per tile
    iota_row = inp("iota_row", [P, P], bf16)      # const 0..127 rows
    iota_col = inp("iota_col", [P, 1], bf16)      # const column 0..127

    Wl1T = inp("Wl1T", [in_ch, hc1], f32r); Wr1T = inp("Wr1T", [in_ch, hc1], f32r)
    P1T = inp("P1T", [in_ch, hc1], f32r)
    bl1_bc = inp("bl1_bc", [P, hc1]); br1_bc = inp("br1_bc", [P, hc1])
    pb1m1_bc = inp("pb1m1_bc", [P, hc1])
    att1_bc = inp("att1_bc", [P, hc1], bf16); bias1_bc = inp("bias1_bc", [P, hc1])

    Wl2T = inp("Wl2T", [hc1, hc2], f32r); Wr2T = inp("Wr2T", [hc1, hc2], f32r)
    P2T = inp("P2T", [hc1, hc2], f32r)
    bl2_bc = inp("bl2_bc", [P, hc2]); br2_bc = inp("br2_bc", [P, hc2])
    pb2m1_bc = inp("pb2m1_bc", [P, hc2])
    att2_bc = inp("att2_bc", [P, hc2], bf16); bias2_bc = inp("bias2_bc", [P, hc2])

    WfT = inp("WfT", [hc2, out_ch]); bf_bc = inp("bf_bc", [P, out_ch])

    out_own = nc.dram_tensor("out_own", [npc, out_ch], f32,
                             kind="ExternalOutput")
    if dbg:
        dbg_xl1 = nc.dram_tensor("dbg_xl1", [128, NB * hc1], f32,
                                 kind="ExternalOutput")
        dbg_xrm1 = nc.dram_tensor("dbg_xrm1", [128, NB * (hc1 + HEADS)], f32,
                                  kind="ExternalOutput")
        dbg_id1 = nc.dram_tensor("dbg_id1", [128, NB * hc1], f32,
                                 kind="ExternalOutput")
        dbg_h = nc.dram_tensor("dbg_h", [128, NB * hc1], f32,
                               kind="ExternalOutput")
        dbg_xl1f = nc.dram_tensor("dbg_xl1f", [N, hc1], f32,
                                  kind="ExternalOutput")

    xl1_own = nc.dram_tensor("xl1_own", [npc, hc1], f32)
    xl1_full = nc.dram_tensor("xl1_full", [N, hc1], f32, addr_space="Shared")
    xl2_own = nc.dram_tensor("xl2_own", [npc, hc2], f32)
    xl2_full = nc.dram_tensor("xl2_full", [N, hc2], f32, addr_space="Shared")

    with tile.TileContext(nc) as tc:
        with (
            tc.tile_pool(name="const", bufs=1) as cp,
            tc.tile_pool(name="res", bufs=1) as rp,
            tc.tile_pool(name="work", bufs=3) as wp_,
            tc.tile_pool(name="gath", bufs=4) as gpool,
        ):
            # ---- constants into SBUF
            ident = cp.tile([P, P], f32)
            make_identity(nc, ident[:])
            with tc.tile_pool(name="pwarm", bufs=1, space="PSUM") as pw:
                warm = pw.tile([P, P], f32)
                nc.tensor.transpose(out=warm[:], in_=ident[:],
                                    identity=ident[:])

            def load_const(t, shape):
                s = cp.tile(shape, t.dtype, tag=f"c_{t.name}")
                nc.sync.dma_start(out=s[:], in_=t[:, :])
                return s

            iota_sb = load_const(iota_row, [P, P])
            iotac_sb = load_const(iota_col, [P, 1])
            w1 = {k: load_const(v, [in_ch, hc1])
                  for k, v in (("wl", Wl1T), ("wr", Wr1T), ("p", P1T))}
            c1b = {k: load_const(v, [P, hc1]) for k, v in (
                ("bl", bl1_bc), ("br", br1_bc), ("pbm1", pb1m1_bc),
                ("att", att1_bc), ("bias", bias1_bc))}
            def load_w2(t):
                # [hc1, hc2] DRAM -> [128, 2*hc2] SBUF (chunk c at col c*hc2)
                s = cp.tile([P, 2 * hc2], t.dtype, tag=f"c_{t.name}")
                for c in range(2):
                    nc.sync.dma_start(out=s[:, c * hc2:(c + 1) * hc2],
                                      in_=t[c * P:(c + 1) * P, :])
                return s

            w2 = {k: load_w2(v) for k, v in (("wl", Wl2T), ("wr", Wr2T),
                                             ("p", P2T))}
            c2b = {k: load_const(v, [P, hc2]) for k, v in (
                ("bl", bl2_bc), ("br", br2_bc), ("pbm1", pb2m1_bc),
                ("att", att2_bc), ("bias", bias2_bc))}
            wf_sb = load_const(WfT, [hc2, out_ch])
            bf_sb = load_const(bf_bc, [P, out_ch])

            xT_sb = rp.tile([in_ch, npc], f32r)
            nc.sync.dma_start(out=xT_sb[:], in_=xT[:, :])
            src1_sb = rp.tile([P, ntt], i32)
            nc.sync.dma_start(out=src1_sb[:], in_=src1[:, :])
            dst1_sb = rp.tile([P, ntt], bf16)
            nc.sync.dma_start(out=dst1_sb[:], in_=dst1[:, :])

            # ---- persistent per-layer node tensors (SBUF resident)
            xl1_sb = rp.tile([P, NB * hc1], f32)    # own xl (self-loop term)
            xrm1_sb = rp.tile([P, NB * (hc1 + HEADS)], bf16)
            id1_sb = rp.tile([P, NB * hc1], f32)    # identity - 1
            h_sb = rp.tile([P, NB * hc1], f32)      # layer-1 output
            xl2_sb = rp.tile([P, NB * hc2], f32)
            xrm2_sb = rp.tile([P, NB * (hc2 + HEADS)], bf16)
            id2_sb = rp.tile([P, NB * hc2], f32)

            def node_ph1(b, ps):
                """layer-1 node phase for block b."""
                lhsT = xT_sb[:, b * B:(b + 1) * B]
                pxl = ps.tile([P, hc1], f32, tag="npl")
                pxr = ps.tile([P, hc1], f32, tag="npr")
                pid = ps.tile([P, hc1], f32, tag="npi")
                nc.tensor.matmul(out=pxl[:B], lhsT=lhsT, rhs=w1["wl"][:],
                                 start=True, stop=True)
                nc.tensor.matmul(out=pxr[:B], lhsT=lhsT, rhs=w1["wr"][:],
                                 start=True, stop=True)
                nc.tensor.matmul(out=pid[:B], lhsT=lhsT, rhs=w1["p"][:],
                                 start=True, stop=True)
                xl = xl1_sb[:B, b * hc1:(b + 1) * hc1]
                nc.vector.tensor_add(out=xl, in0=pxl[:B], in1=c1b["bl"][:B])
                xr = xrm1_sb[:B, b * (hc1 + HEADS):b * (hc1 + HEADS) + hc1]
                nc.vector.tensor_add(out=xr, in0=pxr[:B], in1=c1b["br"][:B])
                nc.vector.tensor_add(
                    out=id1_sb[:B, b * hc1:(b + 1) * hc1],
                    in0=pid[:B], in1=c1b["pbm1"][:B])
                # m = att . lrelu(xl + xr)
                t0 = wp_.tile([P, hc1], f32, tag="nt0")
                nc.vector.tensor_add(out=t0[:B], in0=xl, in1=xr)
                nc.scalar.activation(t0[:B], t0[:B], AF.Prelu, alpha=NEG_SLOPE)
                nc.vector.tensor_mul(out=t0[:B], in0=t0[:B], in1=c1b["att"][:B])
                m = xrm1_sb[:B, b * (hc1 + HEADS) + hc1:(b + 1) * (hc1 + HEADS)]
                mf = wp_.tile([P, HEADS], f32, tag="mf")
                nc.vector.reduce_sum(
                    out=mf[:B], in_=t0[:B].rearrange("p (h c) -> p h c", h=HEADS),
                    axis=mybir.AxisListType.X)
                nc.vector.tensor_copy(out=m, in_=mf[:B])

            with tc.tile_pool(name="pnode1", bufs=1, space="PSUM") as psn:
                for b in range(NB):
                    node_ph1(b, psn)
            # own xl -> DRAM (AllGather input), [B, NB, hc1] -> [npc, hc1]
            nc.sync.dma_start(
                out=xl1_own.ap().rearrange("(b p) c -> p b c", p=B),
                in_=xl1_sb[:B].rearrange("p (b c) -> p b c", b=NB))
            nc.gpsimd.collective_compute(
                "AllGather", ALU.bypass,
                replica_groups=[list(range(NC))],
                ins=[xl1_own.ap().opt()], outs=[xl1_full.ap().opt()])

            # ---- generic edge phase
            def edge_phase(layer, b, src_sb, dst_sb, table, xl_own_sb, xrm_sb,
                           id_sb, hout_sb, hc, consts, ps, accp):
                hcp = hc + HEADS
                seg = accp.tile([P, hcp], f32, tag=f"seg{layer}")
                for k in range(NT):
                    g = b * NT + k
                    xlg = gpool.tile([P, hc], f32, tag="xlg")
                    nc.gpsimd.indirect_dma_start(
                        out=xlg[:], out_offset=None, in_=table.ap(),
                        in_offset=bass.IndirectOffsetOnAxis(
                            ap=src_sb[:, g:g + 1], axis=0))
                    oh = wp_.tile([P, P], bf16, tag="oh")
                    nc.vector.tensor_tensor(
                        out=oh[:], in0=dst_sb[:, g:g + 1].to_broadcast([P, P]),
                        in1=iota_sb[:], op=ALU.is_equal)
                    dr = wp_.tile([P, P], bf16, tag="dr")
                    nc.sync.dma_start(
                        out=dr[:],
                        in_=dstrow.ap()[g:g + 1, :].partition_broadcast(P))
                    ohT = wp_.tile([P, P], bf16, tag="ohT")
                    nc.vector.tensor_tensor(
                        out=ohT[:], in0=iotac_sb[:].to_broadcast([P, P]),
                        in1=dr[:], op=ALU.is_equal)
                    xrm_e = ps.tile([P, hcp], f32, tag="xrm_e")
                    nc.tensor.matmul(
                        out=xrm_e[:], lhsT=ohT[:B, :],
                        rhs=xrm_sb[:B, b * hcp:(b + 1) * hcp],
                        start=True, stop=True)
                    tt = wp_.tile([P, hc], f32, tag="tt")
                    nc.vector.tensor_add(out=tt[:], in0=xlg[:],
                                         in1=xrm_e[:, :hc])
                    t_b = wp_.tile([P, hc], bf16, tag="t_b")
                    nc.scalar.activation(t_b[:], tt[:], AF.Prelu,
                                         alpha=NEG_SLOPE)
                    nc.vector.tensor_mul(out=t_b[:], in0=t_b[:],
                                         in1=consts["att"][:])
                    lg = wp_.tile([P, HEADS], f32, tag="lg")
                    nc.vector.reduce_sum(
                        out=lg[:],
                        in_=t_b[:].rearrange("p (h c) -> p h c", h=HEADS),
                        axis=mybir.AxisListType.X)
                    nc.vector.tensor_tensor(out=lg[:], in0=lg[:],
                                            in1=xrm_e[:, hc:hcp],
                                            op=ALU.subtract)
                    wpt = wp_.tile([P, hcp], bf16, tag="wpt")
                    nc.scalar.activation(wpt[:, hc:hcp], lg[:], AF.Exp)
                    nc.vector.tensor_tensor(
                        out=wpt[:, :hc].rearrange("p (h c) -> p h c", h=HEADS),
                        in0=xlg[:].rearrange("p (h c) -> p h c", h=HEADS),
                        in1=wpt[:, hc:hcp].to_broadcast([P, HEADS, hc // HEADS]),
                        op=ALU.mult)
                    nc.tensor.matmul(out=seg[:B], lhsT=oh[:, :B], rhs=wpt[:],
                                     start=(k == 0), stop=(k == NT - 1))
                # finalize block b
                s = wp_.tile([P, HEADS], f32, tag="fs")
                nc.vector.tensor_scalar_add(s[:B], seg[:B, hc:hcp], 1.0)
                rec = wp_.tile([P, HEADS], f32, tag="frec")
                nc.vector.reciprocal(out=rec[:B], in_=s[:B])
                num = wp_.tile([P, hc], f32, tag="fnum")
                nc.vector.tensor_add(out=num[:B], in0=seg[:B, :hc],
                                     in1=xl_own_sb[:B, b * hc:(b + 1) * hc])
                nc.vector.tensor_tensor(
                    out=num[:B].rearrange("p (h c) -> p h c", h=HEADS),
                    in0=num[:B].rearrange("p (h c) -> p h c", h=HEADS),
                    in1=rec[:B].to_broadcast([B, HEADS, hc // HEADS]),
                    op=ALU.mult)
                nc.vector.tensor_add(out=num[:B], in0=num[:B],
                                     in1=consts["bias"][:B])
                neg = wp_.tile([P, hc], f32, tag="fneg")
                nc.vector.tensor_scalar_min(neg[:B], num[:B], 0.0)
                nc.scalar.activation(neg[:B], neg[:B], AF.Exp)
                nc.vector.tensor_scalar_max(num[:B], num[:B], 0.0)
                nc.vector.tensor_add(out=num[:B], in0=num[:B], in1=neg[:B])
                nc.vector.tensor_add(
                    out=hout_sb[:B, b * hc:(b + 1) * hc], in0=num[:B],
                    in1=id_sb[:B, b * hc:(b + 1) * hc])

            with (
                tc.tile_pool(name="pe1", bufs=2, space="PSUM") as pse,
                tc.tile_pool(name="pa1", bufs=2, space="PSUM") as psa,
            ):
                for b in range(NB):
                    edge_phase(1, b, src1_sb, dst1_sb, xl1_full, xl1_sb,
                               xrm1_sb, id1_sb, h_sb, hc1, c1b, pse, psa)

            # ---- layer-2 node phase (from h_sb)
            def node_ph2(b, ps):
                hT = wp_.tile([P, 2 * B], f32r, tag="hT")
                for cchunk in range(2):
                    tps = ps.tile([P, P], f32, tag="hT_ps")
                    nc.tensor.transpose(
                        out=tps[:, :B],
                        in_=h_sb[:B, b * hc1 + cchunk * P:
                                 b * hc1 + (cchunk + 1) * P],
                        identity=ident[:B, :B])
                    nc.scalar.copy(out=hT[:, cchunk * B:(cchunk + 1) * B],
                                   in_=tps[:, :B])
                pxl = ps.tile([P, hc2], f32, tag="np2l")
                pxr = ps.tile([P, hc2], f32, tag="np2r")
                pid = ps.tile([P, hc2], f32, tag="np2i")
                for cchunk in range(2):
                    lhsT = hT[:, cchunk * B:(cchunk + 1) * B]
                    st, sp = (cchunk == 0), (cchunk == 1)
                    cs = slice(cchunk * hc2, (cchunk + 1) * hc2)
                    nc.tensor.matmul(out=pxl[:B], lhsT=lhsT, rhs=w2["wl"][:, cs],
                                     start=st, stop=sp)
                    nc.tensor.matmul(out=pxr[:B], lhsT=lhsT, rhs=w2["wr"][:, cs],
                                     start=st, stop=sp)
                    nc.tensor.matmul(out=pid[:B], lhsT=lhsT, rhs=w2["p"][:, cs],
                                     start=st, stop=sp)
                xl = xl2_sb[:B, b * hc2:(b + 1) * hc2]
                nc.vector.tensor_add(out=xl, in0=pxl[:B], in1=c2b["bl"][:B])
                xr = xrm2_sb[:B, b * (hc2 + HEADS):b * (hc2 + HEADS) + hc2]
                nc.vector.tensor_add(out=xr, in0=pxr[:B], in1=c2b["br"][:B])
                nc.vector.tensor_add(
                    out=id2_sb[:B, b * hc2:(b + 1) * hc2],
                    in0=pid[:B], in1=c2b["pbm1"][:B])
                t0 = wp_.tile([P, hc2], f32, tag="nt2")
                nc.vector.tensor_add(out=t0[:B], in0=xl, in1=xr)
                nc.scalar.activation(t0[:B], t0[:B], AF.Prelu, alpha=NEG_SLOPE)
                nc.vector.tensor_mul(out=t0[:B], in0=t0[:B], in1=c2b["att"][:B])
                m = xrm2_sb[:B, b * (hc2 + HEADS) + hc2:(b + 1) * (hc2 + HEADS)]
                mf = wp_.tile([P, HEADS], f32, tag="mf2")
                nc.vector.reduce_sum(
                    out=mf[:B], in_=t0[:B].rearrange("p (h c) -> p h c", h=HEADS),
                    axis=mybir.AxisListType.X)
                nc.vector.tensor_copy(out=m, in_=mf[:B])

            with tc.tile_pool(name="pnode2", bufs=1, space="PSUM") as psn2:
                for b in range(NB):
                    node_ph2(b, psn2)
            nc.sync.dma_start(
                out=xl2_own.ap().rearrange("(b p) c -> p b c", p=B),
                in_=xl2_sb[:B].rearrange("p (b c) -> p b c", b=NB))
            nc.gpsimd.collective_compute(
                "AllGather", ALU.bypass,
                replica_groups=[list(range(NC))],
                ins=[xl2_own.ap().opt()], outs=[xl2_full.ap().opt()])

            # ---- layer-2 edge phase + FC per block
            if dbg:
                nc.sync.dma_start(out=dbg_xl1.ap(), in_=xl1_sb[:])
                nc.sync.dma_start(out=dbg_xrm1.ap(), in_=xrm1_sb[:])
                nc.sync.dma_start(out=dbg_id1.ap(), in_=id1_sb[:])
                nc.sync.dma_start(out=dbg_h.ap(), in_=h_sb[:])
                nc.sync.dma_start(out=dbg_xl1f.ap(), in_=xl1_full.ap())
            h2_sb = rp.tile([P, NB * hc2], f32)
            with (
                tc.tile_pool(name="pe2", bufs=2, space="PSUM") as pse2,
                tc.tile_pool(name="pa2", bufs=2, space="PSUM") as psa2,
            ):
                for b in range(NB):
                    edge_phase(2, b, src1_sb, dst1_sb, xl2_full, xl2_sb,
                               xrm2_sb, id2_sb, h2_sb, hc2, c2b, pse2, psa2)
            with tc.tile_pool(name="pfc", bufs=2, space="PSUM") as ps:
              for b in range(NB):
                tps = ps.tile([P, P], f32, tag="fc_ps")
                nc.tensor.transpose(
                    out=tps[:, :B], in_=h2_sb[:B, b * hc2:(b + 1) * hc2],
                    identity=ident[:B, :B])
                h2T = wp_.tile([P, B], f32, tag="fcT")
                nc.scalar.copy(out=h2T[:], in_=tps[:, :B])
                pf = ps.tile([P, out_ch], f32, tag="fc_out")
                nc.tensor.matmul(out=pf[:B], lhsT=h2T[:], rhs=wf_sb[:],
                                 start=True, stop=True)
                ob = wp_.tile([P, out_ch], f32, tag="fc_ob")
                nc.vector.tensor_add(out=ob[:B], in0=pf[:B], in1=bf_sb[:B])
                nc.sync.dma_start(out=out_own.ap()[b * B:(b + 1) * B, :],
                                  in_=ob[:B])
    nc.compile()
    return nc


# ----------------------------------------------------------------------------
# host-side sharding / input prep
# ----------------------------------------------------------------------------

def prep_inputs(x, edge_index, weights, N, B, NB, NT_min=1):
    """Returns (per-core in_maps list, NT). weights: dict of np arrays."""
    npc = B * NB
    P = 128
    src = np.asarray(edge_index[0], dtype=np.int64)
    dst = np.asarray(edge_index[1], dtype=np.int64)

    core = dst // npc
    blk = (dst % npc) // B
    loc = (dst % npc) % B

    # per (core, block) edge lists
    counts = np.zeros((NC, NB), dtype=np.int64)
    order = np.lexsort((dst, blk, core))
    src_s, core_s, blk_s, loc_s = (src[order], core[order], blk[order],
                                   loc[order])
    for c in range(NC):
        for b in range(NB):
            counts[c, b] = np.sum((core_s == c) & (blk_s == b))
    tmax = int(counts.max())
    NT = max(NT_min, (tmax + P - 1) // P)
    tblk = NT * P

    in_ch = x.shape[1]
    hc1 = weights["Wl1"].shape[0]
    hc2 = weights["Wl2"].shape[0]
    out_ch = weights["Wf"].shape[0]

    def bc(v):
        return np.tile(np.asarray(v, np.float32)[None, :], (P, 1))

    consts = {
        "iota_row": np.tile(np.arange(P), (P, 1)).astype(NPBF),
        "iota_col": np.arange(P).reshape(P, 1).astype(NPBF),
        "Wl1T": np.ascontiguousarray(weights["Wl1"].T.astype(np.float32)),
        "Wr1T": np.ascontiguousarray(weights["Wr1"].T.astype(np.float32)),
        "P1T": np.ascontiguousarray(weights["P1"].T.astype(np.float32)),
        "bl1_bc": bc(weights["bl1"]), "br1_bc": bc(weights["br1"]),
        "pb1m1_bc": bc(weights["pb1"] - 1.0),
        "att1_bc": bc(weights["att1"].reshape(-1)).astype(NPBF),
        "bias1_bc": bc(weights["bias1"]),
        "Wl2T": np.ascontiguousarray(weights["Wl2"].T.astype(np.float32)),
        "Wr2T": np.ascontiguousarray(weights["Wr2"].T.astype(np.float32)),
        "P2T": np.ascontiguousarray(weights["P2"].T.astype(np.float32)),
        "bl2_bc": bc(weights["bl2"]), "br2_bc": bc(weights["br2"]),
        "pb2m1_bc": bc(weights["pb2"] - 1.0),
        "att2_bc": bc(weights["att2"].reshape(-1)).astype(NPBF),
        "bias2_bc": bc(weights["bias2"]),
        "WfT": np.ascontiguousarray(weights["Wf"].T.astype(np.float32)),
        "bf_bc": bc(weights["bf"]),
    }

    in_maps = []
    for c in range(NC):
        msk = core_s == c
        sc, bc_, lc = src_s[msk], blk_s[msk], loc_s[msk]
        src_arr = np.zeros((NB, tblk), np.int32)
        dst_arr = np.full((NB, tblk), -1.0, np.float32)  # cast to bf16 below
        for b in range(NB):
            mb = bc_ == b
            n = int(mb.sum())
            src_arr[b, :n] = sc[mb]
            dst_arr[b, :n] = lc[mb]
        # [NB, NT, 128] -> [128, NB*NT] with tile column g = b*NT+k
        src_t = src_arr.reshape(NB, NT, P).transpose(2, 0, 1).reshape(P, NB * NT)
        dst_t = dst_arr.reshape(NB, NT, P).transpose(2, 0, 1).reshape(P, NB * NT)
        xo = np.asarray(x[c * npc:(c + 1) * npc], np.float32)
        im = {"xT": np.ascontiguousarray(xo.T),
              "src1": np.ascontiguousarray(src_t),
              "dst1": np.ascontiguousarray(dst_t).astype(NPBF),
              "dstrow": np.ascontiguousarray(dst_t.T).astype(NPBF)}
        im.update(consts)
        in_maps.append(im)
    return in_maps, NT


_CACHE = {}


def _run(x, edge_index, weights, N, B, NB):
    in_maps, NT = prep_inputs(x, edge_index, weights, N, B, NB)
    in_ch = x.shape[1]
    c1 = weights["att1"].shape[1]
    c2 = weights["att2"].shape[1]
    out_ch = weights["Wf"].shape[0]
    key = (N, B, NB, NT, in_ch, c1, c2, out_ch)
    if key not in _CACHE:
        _CACHE[key] = build_program(*key)
    nc = _CACHE[key]
    res = run_bass_kernel_spmd(nc, in_maps, list(range(NC)))
    out = np.concatenate([res.results[c]["out_own"] for c in range(NC)], 0)
    return out.astype(np.float32)


def kernel(x, edge_index, Wl1, bl1, Wr1, br1, att1, bias1, P1, pb1,
           Wl2, bl2, Wr2, br2, att2, bias2, P2, pb2, Wf, bf):
    x = np.asarray(x)
    weights = dict(Wl1=np.asarray(Wl1), bl1=np.asarray(bl1),
                   Wr1=np.asarray(Wr1), br1=np.asarray(br1),
                   att1=np.asarray(att1), bias1=np.asarray(bias1),
                   P1=np.asarray(P1), pb1=np.asarray(pb1),
                   Wl2=np.asarray(Wl2), bl2=np.asarray(bl2),
                   Wr2=np.asarray(Wr2), br2=np.asarray(br2),
                   att2=np.asarray(att2), bias2=np.asarray(bias2),
                   P2=np.asarray(P2), pb2=np.asarray(pb2),
                   Wf=np.asarray(Wf), bf=np.asarray(bf))
    N = x.shape[0]
    assert N == 20000, "hardcoded for the BrainGAT problem size"
    return _run(x, np.asarray(edge_index), weights, N, B=125, NB=20)

